# revision 49
# baseline (speedup 1.0000x reference)
"""Trainium2 Bass kernel for nn_CoordsToNRF.

out[b, p] = atom_nc[b, p] * (AU2KCALMOLA / MAX_NRF) / ||coords[b, I[p]] - coords[b, J[p]]||^2

Strategy (pure data parallel over batch, 8 cores x 128 batches):
  - Layout: batch on partitions, pairs on the free dim ([128, 8128] per core).
  - Pair gather+subtract on the TensorEngine: per xyz component,
        D_c = CT_c.T @ S
    with S [atom, pairs] the static +1/-1 tril selection matrix. fp16 matmuls
    with a TWO-term residual split (C = C0 + C1, C0 = fp16(C),
    C1 = fp16(C - C0)); both terms accumulate into the same PSUM plane with
    the same S, so no scaled-S copy is needed. Coords are uploaded already
    transposed ([atom, comp, batch]) so no on-device transposes.
  - Elementwise chain per 512-pair group, spread across three engines:
        Vector: t1 = D0^2 + D1^2            (custom DVE op, 3 stages)
        Scalar: sq2 = D2^2                   (activation Square)
        GpSimd: r2 = t1 + sq2                (tensor add)
        Vector: out = anc * (1/r2)           (custom DVE op, 8 stages:
            BITWISE_NOT exponent-flip seed u = x*~x in [-4.5,-4], then a
            quadratic minimax polynomial 1/x ~= ~x * q(u), then * anc)
  - atom_nc is pre-scaled by K and sent as fp16 (values in [0.025, 1.28]),
    halving its DMA traffic. Output stays fp32.
"""

import sys

for _p in ("/opt/trn_rl_repo",):
    if _p not in sys.path:
        sys.path.insert(0, _p)

import numpy as np
from contextlib import ExitStack

import concourse.bass as bass
import concourse.bacc as bacc
import concourse.tile as tile
from concourse import mybir
from concourse.bass_utils import run_bass_kernel_spmd

F32 = mybir.dt.float32
F16 = mybir.dt.float16

N_ATOMS = 128
NC2 = N_ATOMS * (N_ATOMS - 1) // 2  # 8128
BATCH = 1024
N_CORES = 8
BPC = BATCH // N_CORES  # 128 batches per core

AU2KCALMOLA = 627.5095 * 0.529177
MAX_NRF = 13036.0
K_CONST = AU2KCALMOLA / MAX_NRF

GROUP = 512  # pairs per group (one PSUM bank of fp32 per component plane)
GROUPS = [(g, min(GROUP, NC2 - g)) for g in range(0, NC2, GROUP)]
PAIR = 1024  # two groups batched for the tail elementwise ops + out DMA
PAIRS = [(p, min(PAIR, NC2 - p)) for p in range(0, NC2, PAIR)]
# smat load chunks: small first chunk so the PE starts ASAP
_S_SIZES = [512, 1536, 2048, 2048, 1984]
SCHUNKS = []
_c = 0
for _w in _S_SIZES:
    SCHUNKS.append((_c, _w))
    _c += _w
assert _c == NC2
SCHUNK_OF = {}  # group start -> (chunk idx, offset)
for _gi, (_g, _f) in enumerate([(g, min(GROUP, NC2 - g)) for g in range(0, NC2, GROUP)]):
    for _ci, (_cs, _cw) in enumerate(SCHUNKS):
        if _cs <= _g and _g + _f <= _cs + _cw:
            SCHUNK_OF[_g] = (_ci, _g - _cs)
            break
    else:
        raise AssertionError(f"group {_g} not inside one smat chunk")
CHUNK = 4096  # atom_nc load chunk
CHUNKS = [(c, min(CHUNK, NC2 - c)) for c in range(0, NC2, CHUNK)]

# Which engine does the t01 = sq0 + sq1 add: "dma" | "gpsimd" | "vector"
ADD_ENGINE = "gpsimd"

_I, _J = np.tril_indices(N_ATOMS, -1)

# ---------------------------------------------------------------------------
# Custom DVE ops
# ---------------------------------------------------------------------------
from concourse.dve_spec import (  # noqa: E402
    AluOp,
    Bin,
    C0,
    C1,
    C2,
    Spec,
    Src0,
    Src1,
    _has_src1,
    lower as dve_lower,
    sq as dve_sq,
)
from concourse.dve_uop import DveOpSpec  # noqa: E402
import concourse.dve_ops as dve_ops_mod  # noqa: E402
from concourse.dve_ops import DveOp  # noqa: E402

# Quadratic minimax for 1/v on v in [4, 4.5] (rel err ~5.1e-5):
#   p(v) = P0 + P1*v + P2*v^2
# The seed u = x * bitcast(~x) lands in [-4.5, -4], and 1/x = bitcast(~x)/u,
# so with q(u) = -p(-u) = -P0 + P1*u - P2*u^2 we get 1/x ~= bitcast(~x)*q(u).
_P0, _P1, _P2 = 0.70710418, -0.16652117, 0.01306048
QA, QB, QC = -_P0, _P1, -_P2  # q(u) = QA + QB*u + QC*u^2


def _register_dve_op(name: str, spec: Spec) -> DveOp:
    existing = {op.name: op for op in dve_ops_mod.OPS}
    if name in existing:
        return existing[name]
    row = max(dve_ops_mod._SUB_OPCODE_FOR_NAME.values()) + 1
    assert row < 0x20, "custom DVE opcode rows exhausted"
    dve_ops_mod._SUB_OPCODE_FOR_NAME[name] = row
    uops = dve_lower(spec, ver="v3")
    sha = DveOpSpec(name=name, opcode=row, uops=uops, rd1_en=_has_src1(spec)).sha("v3")
    op = DveOp(name, spec, subdim=False, uops_sha={"v3": sha})
    dve_ops_mod.OPS.append(op)
    dve_ops_mod.CUSTOM_DVE_SPECS[name] = spec
    return op


# out = Src0 + Src1^2  (Src1 may come from PSUM; only one PSUM input allowed)
ADD_SQ_OP = _register_dve_op(
    "NRF_ADD_SQ",
    Spec(
        body=Src0 + dve_sq(Src1),
        reference=lambda in0, in1, s0, s1, imm2: (
            in0.astype(np.float32)
            + in1.astype(np.float32) * in1.astype(np.float32)
        ),
    ),
)


def _ref_recip_mul(in0, in1, s0, s1, imm2):
    x = in0.astype(np.float32)
    nx = (~x.view(np.int32)).view(np.float32)
    u = x * nx
    q = (imm2 * u + s1) * u + s0
    return (nx * q) * in1.astype(np.float32)


# out = Src1 * approx(1/Src0); s0/s1/imm2 = QA/QB/QC
_nx = Bin(AluOp.BITWISE_NOT, Src0, Src0)
_u = Src0 * _nx
RECIP_MUL_OP = _register_dve_op(
    "NRF_RECIP_MUL",
    Spec(
        body=((_u * C2 + C1) * _u + C0) * _nx * Src1,
        reference=_ref_recip_mul,
    ),
)


# ---------------------------------------------------------------------------
# Program
# ---------------------------------------------------------------------------
def _build_program():
    nc = bacc.Bacc("TRN2", target_bir_lowering=False, debug=False)

    # coords pre-transposed on host: [atom, term, comp, batch], one DMA
    ct_d = nc.dram_tensor("ct01", [N_ATOMS, 2 * 3 * BPC], F16, kind="ExternalInput")
    anc_d = nc.dram_tensor("anc", [BPC, NC2], F16, kind="ExternalInput")
    smat_d = nc.dram_tensor("smat", [N_ATOMS, NC2], F16, kind="ExternalInput")
    out_d = nc.dram_tensor("out", [BPC, NC2], F32, kind="ExternalOutput")

    with tile.TileContext(nc) as tc, ExitStack() as ctx:
        const = ctx.enter_context(tc.tile_pool(name="const", bufs=1))
        work = ctx.enter_context(tc.tile_pool(name="work", bufs=3))
        outp = ctx.enter_context(tc.tile_pool(name="outp", bufs=3))
        ps_d = ctx.enter_context(tc.tile_pool(name="ps_d", bufs=2, space="PSUM"))
        # d2 is the last-freed plane (read by the vector ADD_SQ); give it an
        # extra buffer so its WAR never gates the tensor engine
        ps_d2 = ctx.enter_context(tc.tile_pool(name="ps_d2", bufs=3, space="PSUM"))

        # ---- inputs ----
        # ct c=0 stationaries (both terms) + the small first smat chunk go
        # first so the first matmul can start ASAP; rest follows
        ct_sb = const.tile([N_ATOMS, 2, 3, BPC], F16, tag="ct01")

        def _ct_load(t):
            del t
            nc.sync.dma_start(
                ct_sb[:], ct_d[:, :].rearrange("a (t c b) -> a t c b", t=2, c=3)
            )

        # chunked loads so early groups don't wait on the whole 8k columns;
        # smat gates the PE so its chunks go first, interleaved ~2:1 with anc.
        smat_sb = []
        for ci, (c0, cw) in enumerate(SCHUNKS):
            st = const.tile([N_ATOMS, cw], F16, tag=f"smat{ci}", name=f"smat{ci}")
            smat_sb.append(st)
        anc_sb = []
        for ci, (c0, cw) in enumerate(CHUNKS):
            at = const.tile([BPC, cw], F16, tag=f"anc{ci}", name=f"anc{ci}")
            anc_sb.append(at)
        def _smat_load(si):
            c0, cw = SCHUNKS[si]
            nc.sync.dma_start(smat_sb[si][:], smat_d[:, c0:c0 + cw])

        def _anc_load(ai):
            c0, cw = CHUNKS[ai]
            nc.sync.dma_start(anc_sb[ai][:], anc_d[:, c0:c0 + cw])

        _ct_load(0)      # all ct stationaries, one DMA
        _smat_load(0)    # small first chunk -> first MM can go
        _smat_load(1)
        _anc_load(0)
        _smat_load(2)
        _smat_load(3)
        _anc_load(1)
        _smat_load(4)

        # ---- main loop: one 512-pair group at a time; the reciprocal and
        # out DMA are issued one group LATE (software pipelining) so the
        # in-order vector queue never head-of-line blocks a ready ADD_SQ
        # behind a RECIP that still waits on the gpsimd add
        pending = []

        def _flush_tail(item):
            fgs, ffd, fr2, fo, fh, last_of_pair = item
            fci, foff = fgs // CHUNK, fgs % CHUNK
            nc.vector._custom_dve(
                RECIP_MUL_OP, out=fo[:, fh:fh + ffd], in0=fr2[:, :ffd],
                in1=anc_sb[fci][:, foff:foff + ffd],
                s0=QA, s1=QB, imm2=QC,
            )
            if last_of_pair:  # one out DMA per 1024-pair block
                fps = fgs - fh
                fpw = fh + ffd
                nc.sync.dma_start(out_d[:, fps:fps + fpw], fo[:, :fpw])

        o_pair = None
        for gi, (gs, fd) in enumerate(GROUPS):
            sci, soff = SCHUNK_OF[gs]
            h = gs % PAIR
            if h == 0:
                o_pair = outp.tile([128, PAIR], F32, tag="o", name="o")
            # one PSUM tile (= one bank) per component plane so each frees
            # independently; consumers issue right after their plane's MMs
            dpl = [
                ps_d.tile([128, GROUP], F32, tag="d0", name="d0"),
                ps_d2.tile([128, GROUP], F32, tag="d1", name="d1"),
                ps_d2.tile([128, GROUP], F32, tag="d2", name="d2"),
            ]
            sq0 = work.tile([128, GROUP], F32, tag="sq0")
            sq1 = work.tile([128, GROUP], F32, tag="sq1")
            r02 = work.tile([128, GROUP], F32, tag="r02")
            r2 = work.tile([128, GROUP], F32, tag="r2")
            for c in range(3):
                for t in range(2):
                    nc.tensor.matmul(
                        dpl[c][:, :fd], ct_sb[:, t, c, :],
                        smat_sb[sci][:, soff:soff + fd],
                        start=(t == 0), stop=(t == 1),
                    )
                if c == 0:  # sq0 = D0^2 on scalar (PSUM -> SBUF)
                    nc.scalar.activation(
                        sq0[:, :fd], dpl[0][:, :fd],
                        mybir.ActivationFunctionType.Square, bias=0.0, scale=1.0,
                    )
                elif c == 1:  # sq1 = D1^2 on scalar
                    nc.scalar.activation(
                        sq1[:, :fd], dpl[1][:, :fd],
                        mybir.ActivationFunctionType.Square, bias=0.0, scale=1.0,
                    )
                else:  # r02 = sq0 + D2^2 on vector (one PSUM input)
                    nc.vector._custom_dve(
                        ADD_SQ_OP, out=r02[:, :fd],
                        in0=sq0[:, :fd], in1=dpl[2][:, :fd],
                    )
            # r2 = r02 + sq1 off the PSUM critical path; alternate engines so
            # the gpsimd queue (which also carries sem updates) stays shallow
            if gi % 2 == 0:
                nc.gpsimd.tensor_add(r2[:, :fd], r02[:, :fd], sq1[:, :fd])
            else:
                nc.vector.tensor_add(r2[:, :fd], r02[:, :fd], sq1[:, :fd])
            pending.append((gs, fd, r2, o_pair, h, h + fd >= PAIR or gs + fd >= NC2))
            if len(pending) > 2:
                _flush_tail(pending.pop(0))
        while pending:
            _flush_tail(pending.pop(0))

    nc.compile()
    return nc


_CACHED = None


def _get_program():
    global _CACHED
    if _CACHED is None:
        _CACHED = _build_program()
    return _CACHED


def _host_prep(coords, atom_nc):
    """Split coords into two fp16 residual terms, pre-transpose to
    [atom, comp, batch] per core slice; fold K into atom_nc as fp16."""
    c32 = coords.astype(np.float32)
    c0 = c32.astype(np.float16)
    c1 = (c32 - c0.astype(np.float32)).astype(np.float16)
    anc16 = (atom_nc * np.float32(K_CONST)).astype(np.float16)
    return c0, c1, anc16


def kernel(coords, atom_nc, _trace=False, _trace_kwargs=None):
    coords = np.ascontiguousarray(np.asarray(coords, dtype=np.float32))
    atom_nc = np.ascontiguousarray(np.asarray(atom_nc, dtype=np.float32))
    assert coords.shape == (BATCH, N_ATOMS, 3)
    assert atom_nc.shape == (BATCH, NC2)

    nc = _get_program()
    smat = np.zeros((N_ATOMS, NC2), dtype=np.float16)
    p = np.arange(NC2)
    smat[_I, p] = 1.0
    smat[_J, p] = -1.0
    c0, c1, anc16 = _host_prep(coords, atom_nc)

    in_maps = []
    for core in range(N_CORES):
        b0 = core * BPC
        # [B, A, 3] slice -> [A, term, 3, B] contiguous fp16, single upload
        ct0 = c0[b0:b0 + BPC].transpose(1, 2, 0)
        ct1 = c1[b0:b0 + BPC].transpose(1, 2, 0)
        ct01 = np.ascontiguousarray(np.stack([ct0, ct1], axis=1))
        in_maps.append({
            "ct01": ct01.reshape(N_ATOMS, 2 * 3 * BPC),
            "anc": anc16[b0:b0 + BPC],
            "smat": smat,
        })

    kw = {}
    if _trace:
        kw["trace"] = True
        kw.update(_trace_kwargs or {})
    res = run_bass_kernel_spmd(nc, in_maps, core_ids=list(range(N_CORES)), **kw)
    out = np.concatenate([r["out"] for r in res.results], axis=0)
    if _trace:
        return out, res
    return out


if __name__ == "__main__":
    rng = np.random.default_rng(0)
    coords = (rng.standard_normal((BATCH, N_ATOMS, 3)) * 5.0).astype(np.float32)
    atom_nc = rng.uniform(1.0, 50.0, (BATCH, NC2)).astype(np.float32)
    out = kernel(coords, atom_nc)
    print(out.shape, out.dtype)
